# revision 13
# baseline (speedup 1.0000x reference)
"""
Trainium2 Bass kernel for nn_BidirectionalAntiAttention — v2 (fp8/DoubleRow).

Same algebraic reformulation as v1 (see kernel.py): one plucker per token per
direction via u-folding, Lagrange identity for pair norms, weight products
folded on host.

v2 speed changes (cost-model driven):
  * All big phase-B matmuls run fp8e4 DoubleRow (2 k-tiles per instruction at
    0.5 cycles/row = 4x bf16 throughput).  x@Wg1 keeps bf16-grade accuracy via
    a 3-term residual split x_hi@W_hi + x_hi@W_lo + x_lo@W_hi2 (all fp8,
    host-split, shared-scale PSUM accumulation).
  * q is produced directly in fp8 at 16x scale (16 folded into the plucker
    gather matrices; the 1/512 descale folded into ACT scale params).
  * Elementwise ops are bf16-in/bf16-out in SBUF so DVE runs in 2x (TT) or 4x
    (copy/TSP) mode; PSUM-reading products go to the otherwise-idle Pool
    engine; ACT ops are grouped into 3 act-table epochs (LnExp|Sigmoid|LnExp)
    so only 3 table loads happen.
  * h and the output are bf16; `scale` is applied on the host.

Backend legality notes (walrus rejects what the cost model allows):
GPSIMD/Pool cannot touch PSUM and only runs plain TensorTensor; DVE ops may
read at most one PSUM operand; fp8 DoubleRow LdWeights needs 16B-aligned even
pair strides. PSUM rings: p1 (z+stats) / pa (plucker) / pb (phase-B+rms),
split so tile-1's stats don't queue behind tile-0's plucker allocations.

Measured (fake-nrt device, 8 cores): max-rel error ~9.5e-3 vs the fp32
reference (harness gate 2e-2); cost-model makespan ~84.2us vs 115.4us for the
bf16 v1 kernel.
"""

import sys

import numpy as np

for _p in ("/opt/trn_rl_repo",):
    if _p not in sys.path:
        sys.path.insert(0, _p)

import ml_dtypes  # noqa: E402

import concourse.bacc as bacc  # noqa: E402
import concourse.mybir as mybir  # noqa: E402
import concourse.tile as tile  # noqa: E402
from concourse.bass_utils import run_bass_kernel_spmd  # noqa: E402

# ---------------------------------------------------------------- constants
B, L, D, R = 4, 2048, 768, 32
OFFS = (1, 2, 4, 8)
NDELT = len(OFFS)
P = R * (R - 1) // 2  # 496
NCORES = 8
TOK = (B * L) // NCORES  # 1024
NT = 512
NTILES = TOK // NT  # 2
HALO = 8
EXT = TOK + 2 * HALO  # 1040
NW = NT + HALO  # 520
NZ = NT + 2 * HALO  # 528
PT = 124
NPT = 4
DK = D // 128  # 6
F32 = mybir.dt.float32
BF16 = mybir.dt.bfloat16
F8 = mybir.dt.float8e4
AF = mybir.ActivationFunctionType
ALU = mybir.AluOpType
DR = mybir.MatmulPerfMode.DoubleRow
BF = ml_dtypes.bfloat16
F8NP = ml_dtypes.float8_e4m3

SQ = 16.0    # fp8 scale on q (folded into gather matrices)
SW = 32.0    # fp8 scale on W_dr / q-weights / wg1hi2
SG1 = 512.0  # fp8 scale on wg1 hi/lo; = SQ*SW = shared PSUM scale

IU0, IU1 = np.triu_indices(R, k=1)

_cache = {}


def _f8(a):
    return np.ascontiguousarray(np.asarray(a, np.float32)).astype(F8NP)


def _bf(a):
    return np.ascontiguousarray(np.asarray(a, np.float32)).astype(BF)


def _kpack(w, kt, psz):
    """[K, M] -> [psz, kt, M] float32 with [p, k, c] = w[k*psz+p, c]."""
    K, M = w.shape
    assert K == kt * psz, (K, kt, psz)
    out = np.empty((psz, kt, M), np.float32)
    for k in range(kt):
        out[:, k, :] = w[k * psz:(k + 1) * psz, :]
    return out


# ---------------------------------------------------------------- host prep
def _derived(W_dr, b_dr, Wf, bf, Wb, bb, Wg, bg, scale):
    f4 = np.float32
    Wg1 = np.asarray(Wg[:D], f4)
    Wg2 = np.asarray(Wg[D:2 * D], f4)
    Wg3 = np.asarray(Wg[2 * D:], f4)
    d = {}

    wdr4 = np.tile(np.asarray(W_dr, f4), (1, NDELT))  # (768, 128)
    d["wdr8"] = _f8(SW * _kpack(wdr4, DK, 128))
    whi = _f8(SG1 * _kpack(Wg1, DK, 128))
    d["whi8"] = whi
    d["wlo8"] = _f8(SG1 * _kpack(Wg1, DK, 128) - whi.astype(f4))

    d["wf28"] = _f8(SW * _kpack(Wf @ Wg2, NPT, PT))
    d["wb38"] = _f8(SW * _kpack(Wb @ Wg3, NPT, PT))
    d["wgcf8"] = _f8(SW * _kpack(0.5 * np.asarray(Wf, f4), NPT, PT))
    d["wgcb8"] = _f8(SW * _kpack(0.5 * np.asarray(Wb, f4), NPT, PT))

    G0 = np.zeros((R, P), f4)
    G1 = np.zeros((R, P), f4)
    G0[IU0, np.arange(P)] = 1.0
    G1[IU1, np.arange(P)] = 1.0
    d["gpk"] = _bf(np.concatenate([SQ * G0, SQ * G1], axis=1))  # [32, 992]

    r4 = np.zeros((128, 128), f4)
    b4 = np.zeros((128, 128), f4)
    for g in range(NDELT):
        r4[32 * g, 32 * g:32 * g + 32] = 1.0
        b4[32 * g:32 * g + 32, 32 * g] = 1.0
    d["spk"] = _bf(
        np.concatenate(
            [np.tile(G0, (NDELT, 1)), np.tile(G1, (NDELT, 1)), r4, b4,
             np.ones((128, 1), f4)],
            axis=1,
        )
    )  # [128, 2*496+257]

    bdr = np.tile(np.asarray(b_dr, f4), NDELT).reshape(128, 1)
    bias_a = np.asarray(bg, f4) + np.asarray(bf, f4) @ Wg2 + \
        np.asarray(bb, f4) @ Wg3
    biasgc = 0.5 * (np.asarray(bf, f4) + np.asarray(bb, f4))
    d["cf32"] = np.ascontiguousarray(
        np.concatenate(
            [bdr, -bias_a.reshape(DK, 128).T, biasgc.reshape(DK, 128).T],
            axis=1,
        ),
        f4,
    )  # [128, 13]
    d["on1"] = _bf(np.ones((1, 128), f4))
    return d


def _shard_arrays(x):
    f4 = np.float32
    x = np.asarray(x, f4)
    shards = []
    for c in range(NCORES):
        b = c // 2
        s0 = (c % 2) * TOK
        xi = x[b, s0:s0 + TOK, :].T  # (768, 1024) fp32
        xe = np.zeros((D, EXT), f4)   # interior + halos for z windows
        lo = s0 - HALO
        a0, b0 = max(lo, 0), min(lo + EXT, L)
        xe[:, a0 - lo:b0 - lo] = x[b, a0:b0, :].T
        xh = _f8(xe)
        xl = _f8(xi - _f8(xi).astype(f4))  # unscaled fp8 residual
        xb = _kpack(xi, DK, 128).reshape(128, DK * TOK)

        tglob = s0 + np.arange(TOK)
        vf = np.stack([(tglob + dl) <= (L - 1) for dl in OFFS]).astype(f4)
        vb = np.stack([(tglob - dl) >= 0 for dl in OFFS]).astype(f4)
        cf = np.maximum(vf.sum(0), 1.0)
        cb = np.maximum(vb.sum(0), 1.0)
        mfs = np.zeros((128, TOK), f4)
        mbs = np.zeros((128, TOK), f4)
        for g in range(NDELT):
            mfs[32 * g] = vf[g] / cf
            mbs[32 * g] = vb[g] / cb

        shards.append(
            {
                "xh8": np.ascontiguousarray(_kpack(xh, DK, 128).astype(F8NP)),
                "xl8": np.ascontiguousarray(_kpack(xl, DK, 128).astype(F8NP)),
                "xb16": _bf(xb),
                "mfb": _bf(np.concatenate([mfs, mbs], axis=1)),
            }
        )
    return shards


# ---------------------------------------------------------------- program
def _build():
    from contextlib import ExitStack

    nc = bacc.Bacc(
        "TRN2",
        target_bir_lowering=False,
        debug=False,
        num_devices=NCORES,
    )

    def din(name, shape, dt=F32):
        return nc.dram_tensor(name, list(shape), dt, kind="ExternalInput").ap()

    xh_d = din("xh8", (128, DK, EXT), F8)
    xl_d = din("xl8", (128, DK, TOK), F8)
    xb_d = din("xb16", (128, DK * TOK), BF16)
    wdr_d = din("wdr8", (128, DK, 128), F8)
    whi_d = din("whi8", (128, DK, D), F8)
    wlo_d = din("wlo8", (128, DK, D), F8)
    wf2_d = din("wf28", (PT, NPT, D), F8)
    wb3_d = din("wb38", (PT, NPT, D), F8)
    wgcf_d = din("wgcf8", (PT, NPT, D), F8)
    wgcb_d = din("wgcb8", (PT, NPT, D), F8)
    gpk_d = din("gpk", (R, 2 * P), BF16)
    spk_d = din("spk", (128, 2 * P + 257), BF16)
    mfb_d = din("mfb", (128, 2 * TOK), BF16)
    cf32_d = din("cf32", (128, 1 + 2 * DK))
    on1_d = din("on1", (1, 128), BF16)

    out_d = nc.dram_tensor("out_t", [D, TOK], BF16, kind="ExternalOutput").ap()

    with tile.TileContext(nc) as tc, ExitStack() as ctx:
        wp = ctx.enter_context(tc.tile_pool(name="weights", bufs=1))
        sp = ctx.enter_context(tc.tile_pool(name="work", bufs=2))
        qp = ctx.enter_context(tc.tile_pool(name="qpool", bufs=2 * NTILES))
        hp = ctx.enter_context(tc.tile_pool(name="hpool", bufs=2 * DK))
        hq = ctx.enter_context(tc.tile_pool(name="hqpool", bufs=2))
        p1 = ctx.enter_context(tc.tile_pool(name="p1", bufs=2, space="PSUM"))
        pa = ctx.enter_context(tc.tile_pool(name="pa", bufs=3, space="PSUM"))
        pb = ctx.enter_context(tc.tile_pool(name="pb", bufs=3, space="PSUM"))

        def wtile(name, dram, queue=None):
            t = wp.tile(list(dram.shape), dram.dtype, name=name)
            (queue or nc.sync).dma_start(t[:], dram[:])
            return t

        # critical path first: tile-0 z window + phase-A consts
        xh = wp.tile([128, DK, EXT], F8, name="xh")
        nc.sync.dma_start(xh[:, :, 0:NZ], xh_d[:, :, 0:NZ])
        wdr = wtile("wdr", wdr_d)
        cf32 = wtile("cf32", cf32_d)
        spk = wtile("spk", spk_d)
        mfb = wtile("mfb", mfb_d)
        gpk = wtile("gpk", gpk_d)
        on1 = wtile("on1", on1_d)
        # bulk loads on the same (SP) queue, after the criticals
        nc.sync.dma_start(xh[:, :, NZ:], xh_d[:, :, NZ:])
        whi = wtile("whi", whi_d)
        wf2 = wtile("wf2", wf2_d)
        xl = wtile("xl", xl_d)
        wlo = wtile("wlo", wlo_d)
        wb3 = wtile("wb3", wb3_d)
        wgcb = wtile("wgcb", wgcb_d)
        wgcf = wtile("wgcf", wgcf_d)
        xb16 = wtile("xb16", xb_d)

        g0 = gpk[:, 0:P]
        g1 = gpk[:, P:2 * P]
        sg0 = spk[:, 0:P]
        sg1 = spk[:, P:2 * P]
        r4 = spk[:, 2 * P:2 * P + 128]
        b4 = spk[:, 2 * P + 128:2 * P + 256]
        onc = spk[:, 2 * P + 256:2 * P + 257]
        bdr = cf32[:, 0:1]
        biasa = cf32[:, 1:1 + DK]
        biasgc = cf32[:, 1 + DK:1 + 2 * DK]
        eps = wp.tile([1, 1], F32, name="eps")
        nc.gpsimd.memset(eps[:], 1e-5)
        warm = wp.tile([1, 1], F32, name="warm")
        nc.scalar.activation(warm[:], eps[:], AF.Sqrt)
        onc8 = wp.tile([128, 2, 16], F8, name="onc8")
        nc.gpsimd.memset(onc8[:], 1.0)
        one32 = wp.tile([128, 1], F32, name="one32")
        nc.gpsimd.memset(one32[:], 1.0)

        def mm(out, lhsT, rhs, start, stop, perf_mode=None):
            """matmul with output free dim chunked to <=512 (PSUM bank).
            For DoubleRow, rhs is 3D [K, 2, n]."""
            n = out.shape[-1]
            o = 0
            while o < n:
                c = min(512, n - o)
                r = rhs[:, :, o:o + c] if perf_mode is not None \
                    else rhs[:, o:o + c]
                nc.tensor.matmul(out[:, o:o + c], lhsT, r,
                                 start=start, stop=stop, perf_mode=perf_mode)
                o += c

        qf_t = [None] * NTILES
        qb_t = [None] * NTILES
        hs_t = [[None] * DK for _ in range(NTILES)]
        ssb_t = [None] * NTILES

        # ================= PHASE A: stats + plucker q (fp8, 16x) ===========
        def phase_a(it):
            tok0 = it * NT

            z_psa = p1.tile([128, NT], F32, name="z_psa", tag="pan")
            z_psb = p1.tile([128, NZ - NT], F32, name="z_psb", tag="pan")
            for kp in range(DK // 2):
                mm(z_psa[:], wdr[:, 2 * kp:2 * kp + 2, :],
                   xh[:, 2 * kp:2 * kp + 2, it * NT:it * NT + NT],
                   kp == 0, kp == DK // 2 - 1, perf_mode=DR)
                mm(z_psb[:], wdr[:, 2 * kp:2 * kp + 2, :],
                   xh[:, 2 * kp:2 * kp + 2, it * NT + NT:it * NT + NZ],
                   kp == 0, kp == DK // 2 - 1, perf_mode=DR)
            z4 = sp.tile([128, NZ], BF16, name="z4", tag="z4", bufs=3)
            nc.scalar.activation(z4[:, 0:NT], z_psa[:], AF.Identity, bias=bdr,
                                 scale=1.0 / SW)
            nc.scalar.activation(z4[:, NT:NZ], z_psb[:], AF.Identity, bias=bdr,
                                 scale=1.0 / SW)
            z = z4[0:R, :]

            z4w = sp.tile([128, NW], BF16, name="z4w", tag="z4w", bufs=3)
            for g, dl in enumerate(OFFS):
                nc.vector.tensor_copy(
                    z4w[32 * g:32 * g + 32, :],
                    z4[32 * g:32 * g + 32, dl:dl + NW],
                )
            z4b = sp.tile([128, NT], BF16, name="z4b", tag="z4b", bufs=3)
            for g, dl in enumerate(OFFS):
                nc.vector.tensor_copy(
                    z4b[32 * g:32 * g + 32, :],
                    z4[32 * g:32 * g + 32, HALO - dl:HALO - dl + NT],
                )

            p4 = sp.tile([128, NW], BF16, name="p4", tag="p4", bufs=3)
            nc.gpsimd.tensor_mul(p4[:], z4[:, 0:NW], z4w[:])
            zw2 = sp.tile([128, NW], BF16, name="zw2", tag="zw2", bufs=3)
            nc.gpsimd.tensor_mul(zw2[:], z4w[:], z4w[:])
            zr2 = sp.tile([128, NW], BF16, name="zr2", tag="zr2", bufs=3)
            nc.vector.tensor_mul(zr2[:], z4[:, 0:NW], z4[:, 0:NW])

            dots_a = p1.tile([128, NT], F32, name="dots_a", tag="pan")
            dots_b = p1.tile([128, NW - NT], F32, name="dots_b", tag="pan")
            nc.tensor.matmul(dots_a[:], b4, p4[:, 0:NT], start=True, stop=True)
            nc.tensor.matmul(dots_b[:], b4, p4[:, NT:NW], start=True, stop=True)
            n4r_a = p1.tile([128, NT], F32, name="n4r_a", tag="pan")
            n4r_b = p1.tile([128, NW - NT], F32, name="n4r_b", tag="pan")
            nc.tensor.matmul(n4r_a[:], b4, zr2[:, 0:NT], start=True, stop=True)
            nc.tensor.matmul(n4r_b[:], b4, zr2[:, NT:NW], start=True, stop=True)
            n2s_a = p1.tile([128, NT], F32, name="n2s_a", tag="pan")
            n2s_b = p1.tile([128, NW - NT], F32, name="n2s_b", tag="pan")
            nc.tensor.matmul(n2s_a[:], b4, zw2[:, 0:NT], start=True, stop=True)
            nc.tensor.matmul(n2s_b[:], b4, zw2[:, NT:NW], start=True, stop=True)

            n4s = sp.tile([128, NW], BF16, name="n4s", tag="n4s", bufs=3)
            nc.scalar.activation(n4s[:, 0:NT], n4r_a[:], AF.Identity)
            nc.scalar.activation(n4s[:, NT:NW], n4r_b[:], AF.Identity)
            nn = sp.tile([128, NW], BF16, name="nn", tag="nn", bufs=3)
            nc.vector.tensor_mul(nn[:, 0:NT], n2s_a[:], n4s[:, 0:NT])
            nc.vector.tensor_mul(nn[:, NT:NW], n2s_b[:], n4s[:, NT:NW])
            d2 = sp.tile([128, NW], BF16, name="d2", tag="d2", bufs=3)
            nc.scalar.activation(d2[:, 0:NT], dots_a[:], AF.Square)
            nc.scalar.activation(d2[:, NT:NW], dots_b[:], AF.Square)
            pn2 = sp.tile([128, NW], BF16, name="pn2", tag="pn2", bufs=3)
            nc.vector.tensor_sub(pn2[:], nn[:], d2[:])
            pn2c = sp.tile([128, NW], BF16, name="pn2c", tag="pn2c", bufs=3)
            nc.vector.tensor_scalar_max(pn2c[:], pn2[:], 1e-16)
            spn = sp.tile([128, NW], BF16, name="spn", tag="spn", bufs=3)
            nc.scalar.activation(spn[:], pn2c[:], AF.Sqrt)
            wraw = sp.tile([128, NW], BF16, name="wraw", tag="wraw", bufs=3)
            with nc.allow_low_precision(reason="bf16 pair weights by design"):
                nc.vector.reciprocal(wraw[:], spn[:])

            w4f = sp.tile([128, NT], BF16, name="w4f", tag="w4f", bufs=2)
            nc.vector.tensor_mul(
                w4f[:], wraw[:, HALO:HALO + NT], mfb[:, tok0:tok0 + NT]
            )
            w4b = sp.tile([128, NT], BF16, name="w4b", tag="w4b", bufs=2)
            nc.gpsimd.memset(w4b[:], 0.0)
            for g, dl in enumerate(OFFS):
                eng = nc.vector if g % 2 == 0 else nc.gpsimd
                eng.tensor_mul(
                    w4b[32 * g:32 * g + 1, :],
                    wraw[32 * g:32 * g + 1, HALO - dl:HALO - dl + NT],
                    mfb[32 * g:32 * g + 1, TOK + tok0:TOK + tok0 + NT],
                )

            wrf_ps = pa.tile([128, NT], F32, name="wrf_ps", tag="pan")
            mm(wrf_ps[:], r4, w4f[:], True, True)
            wrb_ps = pa.tile([128, NT], F32, name="wrb_ps", tag="pan")
            mm(wrb_ps[:], r4, w4b[:], True, True)
            yfb = sp.tile([128, 2 * NT], BF16, name="yfb", tag="yfb", bufs=3)
            nc.vector.tensor_mul(yfb[:, 0:NT], wrf_ps[:],
                                 z4w[:, HALO:HALO + NT])
            nc.vector.tensor_mul(yfb[:, NT:2 * NT], wrb_ps[:], z4b[:])

            qf = qp.tile([PT, NPT, NT], F8, name=f"qf{it}", tag="q")
            qb = qp.tile([PT, NPT, NT], F8, name=f"qb{it}", tag="q")
            qf_t[it], qb_t[it] = qf, qb
            for m in range(NPT):
                sl = slice(PT * m, PT * (m + 1))
                a0_ps = pa.tile([PT, NT], F32, name="a0_ps", tag="pan")
                mm(a0_ps[:], g0[:, sl], z[:, HALO:HALO + NT], True, True)
                a1_ps = pa.tile([PT, NT], F32, name="a1_ps", tag="pan")
                mm(a1_ps[:], g1[:, sl], z[:, HALO:HALO + NT], True, True)
                a0s = sp.tile([PT, NT], BF16, name="a0s", tag="a0s", bufs=4)
                nc.scalar.activation(a0s[:], a0_ps[:], AF.Identity)
                a1s = sp.tile([PT, NT], BF16, name="a1s", tag="a1s", bufs=4)
                nc.scalar.activation(a1s[:], a1_ps[:], AF.Identity)
                for q, half in ((qf, 0), (qb, 1)):
                    a0u_ps = pa.tile([PT, NT], F32, name="a0u_ps", tag="pan")
                    mm(a0u_ps[:], sg0[:, sl],
                       yfb[:, half * NT:(half + 1) * NT], True, True)
                    a1u_ps = pa.tile([PT, NT], F32, name="a1u_ps", tag="pan")
                    mm(a1u_ps[:], sg1[:, sl],
                       yfb[:, half * NT:(half + 1) * NT], True, True)
                    m1 = sp.tile([PT, NT], BF16, name="m1", tag="m1", bufs=8)
                    nc.vector.tensor_mul(m1[:], a1u_ps[:], a0s[:])
                    m2 = sp.tile([PT, NT], BF16, name="m2", tag="m2", bufs=8)
                    nc.vector.tensor_mul(m2[:], a0u_ps[:], a1s[:])
                    with nc.allow_low_precision(reason="fp8 q by design"):
                        nc.gpsimd.tensor_sub(q[:, m, :], m1[:], m2[:])

        # ================= PHASE B: fp8-DR matmuls + combine ===============
        def phase_b(it):
            tok0 = it * NT
            qf, qb = qf_t[it], qb_t[it]
            hsq3 = hq.tile([128, DK, NT], F8, name=f"hsq{it}", tag="hsq",
                           bufs=2)
            for md in range(DK):
                c0, c1 = md * 128, (md + 1) * 128

                al_ps = pb.tile([128, NT], F32, name="al_ps", tag="pan")
                for kp in range(DK // 2):
                    mm(al_ps[:], whi[:, 2 * kp:2 * kp + 2, c0:c1],
                       xh[:, 2 * kp:2 * kp + 2,
                          HALO + tok0:HALO + tok0 + NT],
                       kp == 0, False, perf_mode=DR)
                for kp in range(DK // 2):
                    mm(al_ps[:], wlo[:, 2 * kp:2 * kp + 2, c0:c1],
                       xh[:, 2 * kp:2 * kp + 2,
                          HALO + tok0:HALO + tok0 + NT],
                       False, False, perf_mode=DR)
                for kp in range(DK // 2):
                    mm(al_ps[:], whi[:, 2 * kp:2 * kp + 2, c0:c1],
                       xl[:, 2 * kp:2 * kp + 2, tok0:tok0 + NT],
                       False, False, perf_mode=DR)
                for j in range(NPT // 2):
                    mm(al_ps[:], wf2[:, 2 * j:2 * j + 2, c0:c1],
                       qf[:, 2 * j:2 * j + 2, :], False, False, perf_mode=DR)
                for j in range(NPT // 2):
                    mm(al_ps[:], wb3[:, 2 * j:2 * j + 2, c0:c1],
                       qb[:, 2 * j:2 * j + 2, :], False,
                       j == NPT // 2 - 1, perf_mode=DR)

                gc_ps = pb.tile([128, NT], F32, name="gc_ps", tag="pan")
                for j in range(NPT // 2):
                    mm(gc_ps[:], wgcf[:, 2 * j:2 * j + 2, c0:c1],
                       qf[:, 2 * j:2 * j + 2, :], j == 0, False, perf_mode=DR)
                for j in range(NPT // 2):
                    mm(gc_ps[:], wgcb[:, 2 * j:2 * j + 2, c0:c1],
                       qb[:, 2 * j:2 * j + 2, :], False,
                       j == NPT // 2 - 1, perf_mode=DR)

                s2 = sp.tile([128, NT], BF16, name="s2", tag="s2", bufs=4)
                with tc.tile_wait_until(T_SIGM):
                    nc.scalar.activation(s2[:], al_ps[:], AF.Sigmoid,
                                         bias=biasa[:, md:md + 1],
                                         scale=-1.0 / SG1)
                xbs = xb16[:, md * TOK + tok0:md * TOK + tok0 + NT]
                e = sp.tile([128, NT], BF16, name="e", tag="e", bufs=4)
                gcb = sp.tile([128, NT], BF16, name="gcb", tag="gcb",
                              bufs=4)
                with tc.tile_wait_until(T_SIGM):
                    nc.scalar.activation(gcb[:], gc_ps[:], AF.Identity,
                                         bias=biasgc[:, md:md + 1],
                                         scale=1.0 / SG1)
                nc.vector.tensor_sub(e[:], gcb[:], xbs)
                t = sp.tile([128, NT], BF16, name="t", tag="t", bufs=4)
                nc.vector.tensor_mul(t[:], s2[:], e[:])
                h = hp.tile([128, NT], BF16, name=f"h{it}_{md}", tag="h")
                nc.vector.tensor_add(h[:], xbs, t[:])
                hs_t[it][md] = h
                with nc.allow_low_precision(reason="fp8 h^2 by design"):
                    nc.gpsimd.tensor_mul(hsq3[:, md, :], h[:], h[:])

            ssum_ps = pb.tile([1, NT], F32, name="ssum_ps", tag="pan")
            for j in range(DK // 2):
                mm(ssum_ps[:], onc8[:, :, 0:1], hsq3[:, 2 * j:2 * j + 2, :],
                   j == 0, j == DK // 2 - 1, perf_mode=DR)
            ssb_t[it] = ssum_ps

        # ================= RMS tail ========================================
        def phase_rms(it, t_gate):
            tok0 = it * NT
            sqr = sp.tile([1, NT], F32, name="sqr", tag="sqr", bufs=2)
            rr = sp.tile([1, NT], mybir.dt.float32r, name="rr", tag="rr",
                         bufs=2)
            with tc.tile_wait_until(t_gate):
                nc.scalar.activation(sqr[:], ssb_t[it][:], AF.Sqrt,
                                     scale=1.0 / D, bias=eps[:, 0:1])
            nc.vector.reciprocal(rr[:].bitcast(F32), sqr[:])
            rrep_ps = pb.tile([128, NT], F32, name="rrep_ps", tag="pan")
            mm(rrep_ps[:], on1[:], rr[:], True, True)
            rrb = sp.tile([128, NT], BF16, name="rrb", tag="rrb", bufs=2)
            nc.scalar.activation(rrb[:], rrep_ps[:], AF.Copy)
            for md in range(DK):
                hn = sp.tile([128, NT], BF16, name="hn", tag="hn", bufs=4)
                nc.vector.tensor_mul(hn[:], hs_t[it][md][:], rrb[:])
                nc.sync.dma_start(
                    out_d[128 * md:128 * (md + 1), tok0:tok0 + NT], hn[:]
                )

        T_SIGM = 0.030
        T_RMS0 = 0.2
        T_RMS = 0.2
        # act-table epochs: [ln_exp: A0 A1] [sigmoid: B0 B1] [ln_exp: rms]
        phase_a(0)
        phase_a(1)
        phase_b(0)
        phase_b(1)
        phase_rms(0, T_RMS0)
        phase_rms(1, T_RMS)

    nc.compile()
    return nc


# ---------------------------------------------------------------- entry
def kernel(x, W_dr, b_dr, Wf, bf, Wb, bb, Wg, bg, scale, _run_kwargs=None):
    if "nc" not in _cache:
        _cache["nc"] = _build()
    nc = _cache["nc"]

    shared = _derived(
        np.asarray(W_dr), np.asarray(b_dr), np.asarray(Wf), np.asarray(bf),
        np.asarray(Wb), np.asarray(bb), np.asarray(Wg), np.asarray(bg),
        np.asarray(scale),
    )
    shards = _shard_arrays(np.asarray(x))
    in_maps = [{**shared, **s} for s in shards]

    res = run_bass_kernel_spmd(
        nc, in_maps, core_ids=list(range(NCORES)), **(_run_kwargs or {})
    )
    _cache["last_results"] = res

    scale_f = np.asarray(scale, np.float32)
    out = np.empty((B, L, D), np.float32)
    for c in range(NCORES):
        b = c // 2
        s0 = (c % 2) * TOK
        out[b, s0:s0 + TOK, :] = (
            res.results[c]["out_t"].astype(np.float32).T * scale_f[None, :]
        )
    return out
